# revision 120
# baseline (speedup 1.0000x reference)
"""Multi-head causal attention (B=2, S=2048, D=1024, H=16, DH=64) on 8 NeuronCores.

Sharding: data-parallel over batch (2) x tensor-parallel over heads (4 groups
of 4 heads). Core c handles batch c//4, heads 4*(c%4)..4*(c%4)+3. Each core
computes its head-group's Q/K/V projections, causal softmax attention, and a
partial output projection (Wo row-shard); the host sums the 4 partials per
batch.

Device-side design (vs. a straightforward bf16 kernel):
- Q/K projections run in fp8e4 with the DoubleRow perf mode (256-deep
  contraction per matmul at 0.5 cycles/row). Weight columns are host-permuted
  so the projection PSUM lands partition-aligned with the [32 x 2-ktile]
  per-head layout the fp8 score matmuls consume.
- Scores (K^T Q) are fp8e4 DoubleRow too: head h's operands sit at partition
  base 32h in the staging tiles (bases 0/32/64 are directly addressable;
  head 3 at base 96 is not, so it gets a tiny DMA-shuffled base-0 copy).
- The attention*V matmul uses the exp'd probabilities as the *stationary*
  operand and V (+a ones column) as moving: out free size is 65 instead of
  512, halving AV's PE cost; softmax sums land in column 64 per q-partition.
  Each q-block accumulates as one contiguous PSUM group (interleaved groups
  mis-accumulate on HW), so AV lags the score/exp stream by one head.
- Normalization is a batched reciprocal + one broadcast multiply per (head,
  q-chunk); PE transposes (via identity) rebuild ctxT [dg, q] for Wo.
- The Activation engine (exp) is the pacer at ~81 us busy. The schedule keeps
  it saturated: scores emit one pair ahead of exp; projections for later
  s-chunks and all V-side work are pushed out of the pipe-fill window (qi0's
  AV lags a full q-chunk); DMA triggers are laid out in need-order across the
  SP/Act/Pool DGE queues with tile_wait_until hints so the single shared DMA
  resource serves the critical chain first; the last task interleaves its
  drain (diag pairs exp'd first, per-block AV/norm/transpose/Wo chains).
"""
import numpy as np
import ml_dtypes

import concourse.bass as bass  # noqa: F401
import concourse.mybir as mybir
import concourse.tile as tile
from concourse import bacc
from concourse.bass_utils import run_bass_kernel_spmd

B, S, D, H, DH = 2, 2048, 1024, 16, 64
N_CORES = 8
HPC = 4            # heads per core
DG = HPC * DH      # 256 head dims per core
QW = 512           # q-chunk width
NQ = S // QW       # 4 q-chunks
NKC = S // 128     # 16 k-chunks
NDC = D // 128     # 8 contraction chunks for the bf16 v projection
NC8 = 4            # 256-deep contraction chunks for fp8 projections

BF = mybir.dt.bfloat16
F32 = mybir.dt.float32
F8 = mybir.dt.float8e4
DR = mybir.MatmulPerfMode.DoubleRow

# scale plumbing: wq8 = Wq*0.125*128, wk8 = Wk*128 (host);
# q8 = psum*SCQ = q_raw/2, k8 = psum*SCK = k_raw/4 -> scores = q.k/8
SCQ = 1.0 / 32.0
SCK = 1.0 / 512.0

_CACHE = {}


def _emit(nc):
    xq8d = nc.dram_tensor("xq8", [128, NC8, 2, S], F8, kind="ExternalInput")
    xk8d = nc.dram_tensor("xk8", [128, NC8, 2, S], F8, kind="ExternalInput")
    xvd = nc.dram_tensor("xvT", [D, S], BF, kind="ExternalInput")
    wq8d = nc.dram_tensor("wq8", [128, NC8, 2, DG], F8, kind="ExternalInput")
    wk8d = nc.dram_tensor("wk8", [128, NC8, 2, DG], F8, kind="ExternalInput")
    wvd = nc.dram_tensor("wvT", [D, DG], BF, kind="ExternalInput")
    wod = nc.dram_tensor("woT", [DG, D], BF, kind="ExternalInput")
    mskd = nc.dram_tensor("msk", [128, 4, QW], BF, kind="ExternalInput")
    idnd = nc.dram_tensor("idn", [128, 128], BF, kind="ExternalInput")
    outp = nc.dram_tensor("outp", [S, D], BF, kind="ExternalOutput")

    EXP = mybir.ActivationFunctionType.Exp

    with tile.TileContext(nc) as tc:
        with (
            tc.tile_pool(name="wpool", bufs=1) as wpool,
            tc.tile_pool(name="spool", bufs=1) as spool,
            tc.tile_pool(name="apool", bufs=28) as apool,
            tc.tile_pool(name="cpool", bufs=2) as cpool,
            tc.tile_pool(name="rpool", bufs=4) as rpool,
            tc.tile_pool(name="ppair", bufs=2, space="PSUM") as ppair,
            tc.tile_pool(name="pmain", bufs=2, space="PSUM") as pmain,
            tc.tile_pool(name="pctx", bufs=2, space="PSUM") as pctx,
        ):
            # --- persistent tiles ---
            wq8 = wpool.tile([128, NC8, 2, DG], F8)
            wk8 = wpool.tile([128, NC8, 2, DG], F8)
            wv = wpool.tile([128, NDC, DG], BF)
            wo = wpool.tile([128, 2, D], BF)
            msk = wpool.tile([128, 4, QW], BF)
            idn = wpool.tile([128, 128], BF)
            xq8 = spool.tile([128, NC8, 2, S], F8)
            xk8 = spool.tile([128, NC8, 2, S], F8)
            xv = spool.tile([128, NDC, S], BF)
            qstg = spool.tile([128, 2, S], F8)
            kstg = spool.tile([128, 2, S], F8)
            # heads 0-2 read score operands directly from the staging tiles at
            # base partitions 0/32/64; head 3 (base 96 is not encodable) gets
            # a DMA-shuffled copy at base 0
            q8h3 = spool.tile([32, 2, S], F8)
            k8h3 = spool.tile([32, 2, S], F8)
            vv = spool.tile([128, NKC, HPC, DH + 1], BF)
            ctxT = spool.tile([128, 2, S], BF)

            xv_re = xvd.ap().rearrange("(c p) s -> p c s", p=128)

            # --- input DMAs ---
            # All transfers funnel through one shared DMA resource in request
            # order, so triggers are emitted in need-order: q chain on sync,
            # k chain on scalar (cheap HWDGE), bulk interleaved between
            # shuffle batches.
            nc.sync.dma_start(wq8[:], wq8d.ap())
            nc.scalar.dma_start(wk8[:], wk8d.ap())
            nc.vector.memset(vv[:, :, :, DH : DH + 1], 1.0)

            # --- input DMA order: fast HWDGE queues carry si0/si1 x chunks +
            # small weights; gpsimd (SWDGE) carries the rest interleaved with
            # the shuffle triggers so shared-engine requests land in
            # need-order ---
            nc.sync.dma_start(xq8[:, :, :, 0:QW], xq8d.ap()[:, :, :, 0:QW])
            nc.scalar.dma_start(xk8[:, :, :, 0:QW], xk8d.ap()[:, :, :, 0:QW])
            nc.sync.dma_start(xq8[:, :, :, QW : 2 * QW], xq8d.ap()[:, :, :, QW : 2 * QW])
            nc.scalar.dma_start(xk8[:, :, :, QW : 2 * QW], xk8d.ap()[:, :, :, QW : 2 * QW])
            with tc.tile_wait_until(0.008):
                nc.gpsimd.dma_start(msk[:], mskd.ap())

            def bulk_stage(i):
                # tile_wait_until biases the scheduler (and hence the DGE ring
                # order) so bulk transfers queue at the shared DMA engine in
                # need-order, behind the critical chain and the h3 shuffles
                if i == 1:
                    with tc.tile_wait_until(0.006):
                        nc.gpsimd.dma_start(
                            xq8[:, :, :, 2 * QW : 3 * QW], xq8d.ap()[:, :, :, 2 * QW : 3 * QW]
                        )
                        nc.gpsimd.dma_start(
                            xk8[:, :, :, 2 * QW : 3 * QW], xk8d.ap()[:, :, :, 2 * QW : 3 * QW]
                        )
                elif i == 2:
                    with tc.tile_wait_until(0.008):
                        nc.gpsimd.dma_start(
                            xq8[:, :, :, 3 * QW : S], xq8d.ap()[:, :, :, 3 * QW : S]
                        )
                        nc.gpsimd.dma_start(
                            xk8[:, :, :, 3 * QW : S], xk8d.ap()[:, :, :, 3 * QW : S]
                        )
                        nc.sync.dma_start(idn[:], idnd.ap())
                        nc.sync.dma_start(
                            wo[:], wod.ap().rearrange("(c p) n -> p c n", p=128)
                        )
                elif i == 3:
                    with tc.tile_wait_until(0.011):
                        nc.gpsimd.dma_start(
                            wv[:], wvd.ap().rearrange("(c p) n -> p c n", p=128)
                        )
                        nc.gpsimd.dma_start(xv[:, :, 0:QW], xv_re[:, :, 0:QW])
                    with tc.tile_wait_until(0.014):
                        nc.gpsimd.dma_start(xv[:, :, QW : 2 * QW], xv_re[:, :, QW : 2 * QW])
                elif i == 4:
                    with tc.tile_wait_until(0.026):
                        nc.gpsimd.dma_start(xv[:, :, 2 * QW : 3 * QW], xv_re[:, :, 2 * QW : 3 * QW])
                    with tc.tile_wait_until(0.040):
                        nc.gpsimd.dma_start(xv[:, :, 3 * QW : S], xv_re[:, :, 3 * QW : S])

            # --- Q/K projections: fp8 DoubleRow; si0 up-front, si1-3
            # interleaved into the early attention tasks ---
            def emit_proj(si, which=(0, 1), cols=None):
                ssl = slice(si * QW, (si + 1) * QW) if cols is None else cols
                plist = (
                    (wq8, xq8, qstg, q8h3, SCQ),
                    (wk8, xk8, kstg, k8h3, SCK),
                )
                for w8, x8t, stg, dst, sc in (plist[w] for w in which):
                    for kh in range(2):
                        ps = pmain.tile([128, QW], F32, tag="ps")
                        psl = ps[:, 0 : ssl.stop - ssl.start]
                        for c in range(NC8):
                            nc.tensor.matmul(
                                psl,
                                w8[:, c, :, kh * 128 : (kh + 1) * 128],
                                x8t[:, c, :, ssl],
                                start=(c == 0),
                                stop=(c == NC8 - 1),
                                perf_mode=DR,
                            )
                        with tc.high_priority(offset=96):
                            with nc.allow_low_precision(reason="fp8 score operand storage"):
                                if si == 0 and stg is kstg:
                                    # ACT is idle before the first exp; copy
                                    # there, first k-columns first
                                    hw_ = QW // 2
                                    nc.scalar.mul(stg[:, kh, 0:hw_], psl[:, 0:hw_], sc)
                                    nc.scalar.mul(stg[:, kh, hw_:QW], psl[:, hw_:QW], sc)
                                else:
                                    nc.vector.tensor_scalar_mul(stg[:, kh, ssl], psl, sc)
                    if cols is None or cols.stop == (si + 1) * QW:
                        fsl = slice(si * QW, (si + 1) * QW)
                        # head 3 sits at base partition 96 (not encodable in
                        # an engine AP) - shuffle it down to a base-0 copy
                        eng = nc.sync if stg is qstg else nc.gpsimd
                        eng.dma_start(dst[:, :, fsl], stg[96:128, :, fsl])

            emit_proj(0)

            # --- v projection (bf16, natural [s, dh] into vv) ---
            def emit_vproj(st):
                ps = pmain.tile([128, DG], F32, tag="ps")
                for c in range(NDC):
                    nc.tensor.matmul(
                        ps[:],
                        xv[:, c, st * 128 : (st + 1) * 128],
                        wv[:, c, :],
                        start=(c == 0),
                        stop=(c == NDC - 1),
                    )
                nc.vector.tensor_copy(
                    vv[:, st, :, 0:DH],
                    ps[:].rearrange("p (h e) -> p h e", e=DH),
                )

            dma_out_engines = (nc.sync, nc.gpsimd)

            obuf = spool.tile([128, 4, D], BF)  # 4 rotating output rows

            def emit_wo(qt, tail=False):
                ob = obuf[:, qt % 4, :]
                for nh in range(2):
                    ops = pmain.tile([128, 512], F32, tag="ps")
                    for t in range(2):
                        nc.tensor.matmul(
                            ops[:],
                            ctxT[:, t, qt * 128 : (qt + 1) * 128],
                            wo[:, t, nh * 512 : (nh + 1) * 512],
                            start=(t == 0),
                            stop=(t == 1),
                        )
                    nsl = slice(nh * 512, (nh + 1) * 512)
                    if tail and nh == 0:
                        # ACT is idle once the exp stream has drained; split
                        # the two halves across ACT and DVE
                        nc.scalar.copy(ob[:, nsl], ops[:])
                    else:
                        nc.vector.tensor_copy(ob[:, nsl], ops[:])
                    if tail:
                        # the SP HWDGE queue is cheap and idle at the end
                        eng = nc.sync
                    else:
                        eng = dma_out_engines[(2 * qt + nh) % 2]
                    eng.dma_start(
                        outp.ap()[qt * 128 : (qt + 1) * 128, nsl], ob[:, nsl]
                    )

            # --- attention: flat (qi, h) pipeline, AV lagging one head so the
            # exp stream never waits, with contiguous PSUM accumulation groups
            ctxn_store = {}

            def emit_scores(qi, h):
                ats = []
                for pc in range(2 * (qi + 1)):
                    d0 = 2 * pc - 4 * qi
                    c0 = QW // 2 if d0 == 2 else 0
                    csl = slice(c0, QW)
                    sps = ppair.tile([128, 2, QW], F32, tag="sps")
                    if h == 3:
                        kop, qop = k8h3[:], q8h3[:]
                    else:
                        kop = kstg[32 * h : 32 * h + 32]
                        qop = qstg[32 * h : 32 * h + 32]
                    at = apool.tile([128, 2, QW], BF, tag="at")
                    with tc.high_priority(offset=192 if pc == 0 else 144):
                        for half in range(2):
                            kc = 2 * pc + half
                            nc.tensor.matmul(
                                sps[:, half, csl],
                                kop[:, :, kc * 128 : (kc + 1) * 128],
                                qop[:, :, qi * QW + c0 : (qi + 1) * QW],
                                start=True,
                                stop=True,
                                perf_mode=DR,
                            )
                    with tc.high_priority(offset=192 if pc == 0 else 144):
                        nc.scalar.activation(at[:, :, csl], sps[:, :, csl], EXP)
                    if d0 >= 0:
                        nc.vector.tensor_mul(
                            at[:, :, csl], at[:, :, csl], msk[:, d0 : d0 + 2, csl]
                        )
                    if qi == 1 and h == 1 and pc < 4:
                        emit_vproj(2 * pc)
                        emit_vproj(2 * pc + 1)
                    elif qi >= 2 and h == 0 and pc < 4:
                        emit_vproj(4 * qi + pc)
                    ats.append(at)
                return ats

            def emit_avnorm(qi, h, ats):
                if h == 0:
                    ctxn = cpool.tile([128, 4, HPC, DH], BF, tag="ctxn")
                    ctxn_store[qi] = ctxn
                else:
                    ctxn = ctxn_store[qi]
                cps = pctx.tile([128, 4, DH + 1], F32, tag="cps")
                for qb in range(4):
                    last = 4 * qi + qb
                    for kc in range(last + 1):
                        nc.tensor.matmul(
                            cps[:, qb, :],
                            ats[kc // 2][:, kc % 2, qb * 128 : (qb + 1) * 128],
                            vv[:, kc, h, :],
                            start=(kc == 0),
                            stop=(kc == last),
                        )
                rc = rpool.tile([128, 4, 1], F32)
                nc.vector.reciprocal(rc[:], cps[:, :, DH : DH + 1])
                nc.vector.tensor_tensor(
                    ctxn[:, :, h, :],
                    cps[:, :, 0:DH],
                    rc[:].broadcast_to([128, 4, DH]),
                    mybir.AluOpType.mult,
                )

            def emit_transposes(qi):
                ctxn = ctxn_store.pop(qi)
                for qb in range(4):
                    for j in range(2):
                        trt = pctx.tile([128, 4, DH + 1], F32, tag="cps")
                        trap = trt[:].bitcast(BF).rearrange("p a b -> p (a b)")[:, 0:128]
                        nc.tensor.transpose(trap, ctxn[:, qb, 2 * j : 2 * j + 2, :], idn[:])
                        nc.vector.tensor_copy(
                            ctxT[:, j, qi * QW + qb * 128 : qi * QW + (qb + 1) * 128],
                            trap,
                        )

            def emit_last_task(qi, h):
                # final (qi, h): diagonal (masked) pairs first so the late
                # exps are mask-free; each q-block's accumulation ends on a
                # staggered non-diag chunk so drain chains fire while the
                # last exps still stream
                cps = pctx.tile([128, 4, DH + 1], F32, tag="cps")
                ats = {}
                # pair order: full-width diagonal pair first (its masks hide
                # in the stream), the non-diag pairs, then the small half-width
                # diagonal pair last - so the low q-block AV chains (which
                # don't need it) fire before the final exp
                pcs = [2 * qi] + list(range(2 * qi)) + [2 * qi + 1]
                kop, qop = (
                    (k8h3[:], q8h3[:])
                    if h == 3
                    else (kstg[32 * h : 32 * h + 32], qstg[32 * h : 32 * h + 32])
                )

                def qb_av(qb):
                    order = list(range(4 * qi, 4 * qi + qb + 1)) + list(range(4 * qi))
                    for kc in order:
                        nc.tensor.matmul(
                            cps[:, qb, :],
                            ats[kc // 2][:, kc % 2, qb * 128 : (qb + 1) * 128],
                            vv[:, kc, h, :],
                            start=(kc == order[0]),
                            stop=(kc == order[-1]),
                        )
                    ctxn = ctxn_store[qi]
                    rc = rpool.tile([128, 1, 1], F32)
                    nc.vector.reciprocal(rc[:], cps[:, qb : qb + 1, DH : DH + 1])
                    nc.vector.tensor_tensor(
                        ctxn[:, qb : qb + 1, h, :],
                        cps[:, qb : qb + 1, 0:DH],
                        rc[:].broadcast_to([128, 1, DH]),
                        mybir.AluOpType.mult,
                    )

                def qb_tr(qb):
                    ctxn = ctxn_store[qi]
                    for j in range(2):
                        trt = pctx.tile([128, 4, DH + 1], F32, tag="cps")
                        trap = trt[:].bitcast(BF).rearrange("p a b -> p (a b)")[:, 0:128]
                        nc.tensor.transpose(trap, ctxn[:, qb, 2 * j : 2 * j + 2, :], idn[:])
                        nc.vector.tensor_copy(
                            ctxT[:, j, qi * QW + qb * 128 : qi * QW + (qb + 1) * 128],
                            trap,
                        )

                for pos, pc in enumerate(pcs):
                    d0 = 2 * pc - 4 * qi
                    c0 = QW // 2 if d0 == 2 else 0
                    csl = slice(c0, QW)
                    sps = ppair.tile([128, 2, QW], F32, tag="sps")
                    with tc.high_priority(offset=192 if pos == 0 else 144):
                        for half in range(2):
                            kc = 2 * pc + half
                            nc.tensor.matmul(
                                sps[:, half, csl],
                                kop[:, :, kc * 128 : (kc + 1) * 128],
                                qop[:, :, qi * QW + c0 : (qi + 1) * QW],
                                start=True,
                                stop=True,
                                perf_mode=DR,
                            )
                    at = apool.tile([128, 2, QW], BF, tag="at")
                    with tc.high_priority(offset=192 if pos == 0 else 144):
                        nc.scalar.activation(at[:, :, csl], sps[:, :, csl], EXP)
                    if d0 >= 0:
                        nc.vector.tensor_mul(
                            at[:, :, csl], at[:, :, csl], msk[:, d0 : d0 + 2, csl]
                        )
                    ats[pc] = at
                    if pos == len(pcs) - 2:
                        # all chunks for q-blocks 0/1 are exp'd; start their
                        # accumulation while the last pair streams
                        qb_av(0)
                        qb_av(1)
                qb_av(2)
                qb_av(3)
                for qb in range(4):
                    qb_tr(qb)
                    if qb > 0:
                        emit_wo(4 * qi + qb - 1, tail=True)
                emit_wo(4 * qi + 3, tail=True)
                ctxn_store.pop(qi)

            wo_ready = []
            pending = []
            for qi in range(NQ):
                for h in range(HPC):
                    if qi == NQ - 1 and h == HPC - 1:
                        # flush the lag-1 predecessor, then the fused last task
                        while pending:
                            pqi, ph, pats = pending.pop(0)
                            emit_avnorm(pqi, ph, pats)
                            if ph == HPC - 1:
                                emit_transposes(pqi)
                                wo_ready.extend(range(4 * pqi, 4 * pqi + 4))
                        for qt in wo_ready:
                            emit_wo(qt)
                        wo_ready = []
                        emit_last_task(qi, h)
                        continue
                    ats = emit_scores(qi, h)
                    ti = 4 * qi + h
                    if 1 <= ti <= 3:
                        emit_proj(ti)
                        bulk_stage(ti)
                    elif ti == 4:
                        bulk_stage(4)
                    pending.append((qi, h, ats))
                    # qi0's AV lags a full q-chunk (everything V-side is
                    # deferred out of the pipe-fill window); lag-1 afterwards
                    while pending:
                        pqi, ph, pats = pending[0]
                        # qi0's vv chunks are only emitted in task (1,1)=5
                        ready = (5 + ph) if pqi == 0 else (4 * pqi + ph + 1)
                        if ready > ti:
                            break
                        pending.pop(0)
                        emit_avnorm(pqi, ph, pats)
                        if ph == HPC - 1:
                            emit_transposes(pqi)
                            wo_ready.extend(range(4 * pqi, 4 * pqi + 4))
                    if wo_ready:
                        emit_wo(wo_ready.pop(0))
            while len(pending) > 1:
                pqi, ph, pats = pending.pop(0)
                emit_avnorm(pqi, ph, pats)
                if ph == HPC - 1:
                    emit_transposes(pqi)
                    wo_ready.extend(range(4 * pqi, 4 * pqi + 4))
            for qt in wo_ready:
                emit_wo(qt)


def build_program():
    if "nc" in _CACHE:
        return _CACHE["nc"]
    nc = bacc.Bacc(
        "TRN2", target_bir_lowering=False, debug=False, num_devices=N_CORES
    )
    _emit(nc)
    nc.compile()
    _CACHE["nc"] = nc
    return nc


def _prep_in_maps(query, key, value, Wq, Wk, Wv, Wo):
    bf = ml_dtypes.bfloat16
    f8 = ml_dtypes.float8_e4m3

    p, i, j = np.ogrid[0:128, 0:4, 0:QW]
    msk = (j >= 128 * i + p).astype(bf)
    idn = np.eye(128, dtype=bf)

    # weight column permutation: psum partition m <- local head-group row
    # 64*(m//32) + 32*k + m%32, so the fp8 score operands are partition-aligned
    perm = np.empty((2, 128), dtype=np.int64)
    for k in range(2):
        for m in range(128):
            perm[k, m] = 64 * (m // 32) + 32 * k + (m % 32)

    def pack_w8(w_loc):
        # w_loc: [DG, D] (rows = local head dims, cols = D contraction)
        w2 = w_loc[perm].transpose(2, 0, 1)  # [D, 2, 128]
        return np.ascontiguousarray(
            w2.reshape(NC8, 2, 128, 2, 128)
            .transpose(2, 0, 1, 3, 4)
            .reshape(128, NC8, 2, DG)
        ).astype(f8)

    def pack_x8(x):
        # x: [S, D] -> [128, NC8, 2, S] with d = 256c + 128t + p
        return np.ascontiguousarray(
            x.T.reshape(NC8, 2, 128, S).transpose(2, 0, 1, 3)
        ).astype(f8)

    xq8b, xk8b, xvb = {}, {}, {}
    for b in range(B):
        xq8b[b] = pack_x8(np.asarray(query[b]))
        xk8b[b] = pack_x8(np.asarray(key[b]))
        xvb[b] = np.ascontiguousarray(value[b].T).astype(bf)

    in_maps = []
    for c in range(N_CORES):
        b, g = c // HPC, c % HPC
        rows = slice(g * DG, (g + 1) * DG)
        in_maps.append(
            {
                "xq8": xq8b[b],
                "xk8": xk8b[b],
                "xvT": xvb[b],
                "wq8": pack_w8(Wq[rows] * (0.125 * 128.0)),
                "wk8": pack_w8(Wk[rows] * 128.0),
                "wvT": np.ascontiguousarray(Wv[rows].T).astype(bf),
                "woT": np.ascontiguousarray(Wo[:, rows].T).astype(bf),
                "msk": msk,
                "idn": idn,
            }
        )
    return in_maps


def kernel(query, key, value, Wq, Wk, Wv, Wo):
    query = np.asarray(query, dtype=np.float32)
    key = np.asarray(key, dtype=np.float32)
    value = np.asarray(value, dtype=np.float32)
    Wq = np.asarray(Wq, dtype=np.float32)
    Wk = np.asarray(Wk, dtype=np.float32)
    Wv = np.asarray(Wv, dtype=np.float32)
    Wo = np.asarray(Wo, dtype=np.float32)

    nc = build_program()
    in_maps = _prep_in_maps(query, key, value, Wq, Wk, Wv, Wo)
    res = run_bass_kernel_spmd(
        nc, in_maps, core_ids=list(range(N_CORES)), trace=False
    )
    out = np.zeros((B, S, D), dtype=np.float32)
    for b in range(B):
        for g in range(HPC):
            out[b] += res.results[b * HPC + g]["outp"].astype(np.float32)
    return out
